# revision 1
# baseline (speedup 1.0000x reference)
"""Causal attention (B=4, N=2048, D=1024) on 8 Trainium2 NeuronCores.

Sharding: core 2b+p handles batch b with query tiles {p, p+2, ..., p+14}
(128-row tiles, parity-interleaved).  Every core runs the same program:
8 query slots with key-tile limits (2, 4, ..., 16) — an exactly balanced
causal split.  Per-core masks are passed as input data so the program is
uniform across cores (SPMD).

All matmuls run in float32r (TF32-like, full PE rate at N>=256); fp32
arrays are fed bit-identically into float32r DRAM params (HW rounds at
the PE input).  x is pre-transposed on the host into d-major tile layout
so no on-chip transposes are needed for the projections.

Schedule: Q^T is computed first and spilled to DRAM; then keys are
processed in two halves (V + K^T into SBUF-resident tiles), with
attention slots 0-3 placed between the halves so the scheduler can
overlap early attention with the second half's projections.  Softmax is
single-pass over the full key row (<= 4 PSUM banks) with exp + row-sum
fused on the scalar engine.
"""
import sys

sys.path.insert(0, "/opt/trn_rl_repo")

from contextlib import ExitStack

import numpy as np

import concourse.bass as bass
import concourse.mybir as mybir
import concourse.tile as tile
from concourse import bacc
from concourse.bass_utils import run_bass_kernel_spmd
from concourse.masks import make_identity

B, N, D = 4, 2048, 1024
N_CORES = 8
N_SLOTS = 8          # query tiles per core
N_KTILES = 16        # 128-key tiles per batch
SCALE = 1.0 / 32.0   # 1/sqrt(D)
NEG = -1.0e9

F32 = mybir.dt.float32
F32R = mybir.dt.float32r

_NC_CACHE = {}
TRACE = False
LAST_EXEC_NS = None


def _build_nc():
    nc = bacc.Bacc(None, target_bir_lowering=False, debug=False)

    # x pre-transposed on host: [tile, partition(d%128), dchunk, token]
    x_t = nc.declare_dram_parameter("x_t", [N_KTILES, 128, 8, 128], F32R, isOutput=False)
    x_qt = nc.declare_dram_parameter("x_qt", [N_SLOTS, 128, 8, 128], F32R, isOutput=False)
    # weights host-rearranged: wq/wk [echunk, p(d%128), dchunk, ecol]; wv [eh, p, dchunk, ecol]
    wq = nc.declare_dram_parameter("wq", [8, 128, 8, 128], F32R, isOutput=False)
    wk = nc.declare_dram_parameter("wk", [8, 128, 8, 128], F32R, isOutput=False)
    wv = nc.declare_dram_parameter("wv", [2, 128, 8, 512], F32R, isOutput=False)
    mask_in = nc.declare_dram_parameter("mask", [128, 256], F32, isOutput=False)
    out_q = nc.declare_dram_parameter("out_q", [N_SLOTS, 128, D], F32, isOutput=True)

    # DRAM scratch: Q^T per-slot-contiguous, V spill for key tiles 13..15
    qt_spill = nc.dram_tensor("qt_spill", [N_SLOTS, 128, 8, 128], F32R, kind="Internal")
    v_spill = nc.dram_tensor("v_spill", [2, 128, D], F32R, kind="Internal")

    with tile.TileContext(nc) as tc, ExitStack() as top:
        consts = top.enter_context(tc.tile_pool(name="consts", bufs=1))
        kt_pool = top.enter_context(tc.tile_pool(name="ktp", bufs=1))
        v_pool = top.enter_context(tc.tile_pool(name="vp", bufs=1))
        qt_pool2 = top.enter_context(tc.tile_pool(name="qtl", bufs=2))

        ident_f = consts.tile([128, 128], F32)
        make_identity(nc, ident_f)
        ident = consts.tile([128, 128], F32R)
        nc.vector.tensor_copy(ident, ident_f)
        mask_sb = consts.tile([128, 256], F32)
        nc.sync.dma_start(out=mask_sb, in_=mask_in[:, :])

        KT = kt_pool.tile([128, 8, N], F32R)      # [p(e%128), echunk, key]
        V = v_pool.tile([128, 14, D], F32R)

        with ExitStack() as ph12:
            xt_pool = ph12.enter_context(tc.tile_pool(name="xtp", bufs=1))
            wv_pool = ph12.enter_context(tc.tile_pool(name="wvp", bufs=2))
            we_pool = ph12.enter_context(tc.tile_pool(name="wep", bufs=2))
            qst_pool = ph12.enter_context(tc.tile_pool(name="qst", bufs=1))
            ps_mm = ph12.enter_context(tc.tile_pool(name="ps_mm", bufs=8, space="PSUM"))

            def project_keys(kh):
                """V and K^T for key tiles kh*8 .. kh*8+7."""
                xT = xt_pool.tile([128, 8, 8, 128], F32R, tag="xT", name=f"xk{kh}")
                for lt in range(8):
                    t = kh * 8 + lt
                    nc.gpsimd.dma_start(out=xT[:, lt, :, :], in_=x_t[t][:, :, :])
                for eh in range(2):
                    wv_sb = wv_pool.tile([128, 8, 512], F32R, tag="wv", name=f"wv{kh}_{eh}")
                    for h2 in range(2):
                        nc.scalar.dma_start(
                            out=wv_sb[:, h2 * 4:(h2 + 1) * 4, :],
                            in_=wv[eh][:, h2 * 4:(h2 + 1) * 4, :],
                        )
                    for lt in range(8):
                        t = kh * 8 + lt
                        vps = ps_mm.tile([128, 512], F32, tag="mm", name=f"v{kh}_{eh}_{lt}")
                        for c in range(8):
                            nc.tensor.matmul(
                                vps, xT[:, lt, c, :], wv_sb[:, c, :],
                                start=(c == 0), stop=(c == 7),
                            )
                        if t < 14:
                            nc.vector.tensor_copy(V[:, t, eh * 512:(eh + 1) * 512], vps)
                        else:
                            vst = qst_pool.tile([128, 512], F32R, tag="qs", name=f"vs{t}_{eh}")
                            nc.vector.tensor_copy(vst, vps)
                            nc.sync.dma_start(
                                out=v_spill[t - 14][:, eh * 512:(eh + 1) * 512], in_=vst
                            )
                for e in range(8):
                    wk_sb = we_pool.tile([128, 8, 128], F32R, tag="we", name=f"wk{kh}_{e}")
                    nc.scalar.dma_start(out=wk_sb, in_=wk[e][:, :, :])
                    kps = [ps_mm.tile([128, 512], F32, tag="mm", name=f"k{kh}_{e}_{g}")
                           for g in range(2)]
                    for c in range(8):
                        for kg in range(2):
                            nc.tensor.matmul(
                                kps[kg], wk_sb[:, c, :], xT[:, kg * 4:(kg + 1) * 4, c, :],
                                start=(c == 0), stop=(c == 7),
                            )
                    for kg in range(2):
                        nc.vector.tensor_copy(
                            KT[:, e, (kh * 2 + kg) * 512:(kh * 2 + kg + 1) * 512], kps[kg]
                        )

            def project_queries():
                xT = xt_pool.tile([128, 8, 8, 128], F32R, tag="xT", name="xq")
                for s in range(N_SLOTS):
                    nc.gpsimd.dma_start(out=xT[:, s, :, :], in_=x_qt[s][:, :, :])
                for e in range(8):
                    wq_sb = we_pool.tile([128, 8, 128], F32R, tag="we", name=f"wq{e}")
                    nc.scalar.dma_start(out=wq_sb, in_=wq[e][:, :, :])
                    qps = [ps_mm.tile([128, 512], F32, tag="mm", name=f"q{e}_{g}")
                           for g in range(2)]
                    for c in range(8):
                        for qg in range(2):
                            nc.tensor.matmul(
                                qps[qg], wq_sb[:, c, :], xT[:, qg * 4:(qg + 1) * 4, c, :],
                                start=(c == 0), stop=(c == 7),
                            )
                    qstage = qst_pool.tile([128, 1024], F32R, tag="qs", name=f"qs{e}")
                    for qg in range(2):
                        nc.vector.tensor_copy(qstage[:, qg * 512:(qg + 1) * 512], qps[qg])
                    nc.sync.dma_start(
                        out=qt_spill[:, :, e, :].rearrange("s p q -> p s q"),
                        in_=qstage.rearrange("p (s q) -> p s q", s=8),
                    )

            project_keys(0)
            project_queries()  # qt spill roundtrip + kh1 x loads hide here
            project_keys(1)

        # ---- attention slots 0-7, software-pipelined AV ----
        with ExitStack() as ph3:
            ps_tr = ph3.enter_context(tc.tile_pool(name="ps_tr", bufs=2, space="PSUM"))
            ps_o = ph3.enter_context(tc.tile_pool(name="ps_o", bufs=1, space="PSUM"))
            p_hi = ph3.enter_context(tc.tile_pool(name="phi", bufs=2))
            pt_pool = ph3.enter_context(tc.tile_pool(name="ptp", bufs=2))
            sc_pool = ph3.enter_context(tc.tile_pool(name="scp", bufs=2))
            outp = ph3.enter_context(tc.tile_pool(name="outp", bufs=2))
            vh_pool = ph3.enter_context(tc.tile_pool(name="vhp", bufs=1))
            v_hi = []

            def emit_av(i, L, P_sb, recip):
                O_ps = ps_o.tile([128, D], F32, tag="O", name=f"O{i}")
                for kt in range(L):
                    ptps = ps_tr.tile([128, 128], F32R, tag="tr", name=f"tp{i}_{kt}")
                    nc.tensor.transpose(ptps, P_sb[:, kt * 128:(kt + 1) * 128], ident)
                    pt_sb = pt_pool.tile([128, 128], F32R, tag="pts", name=f"pt{i}_{kt}")
                    nc.vector.tensor_copy(pt_sb, ptps)
                    vsrc = V[:, kt, :] if kt < 14 else v_hi[kt - 14]
                    for h in range(2):
                        nc.tensor.matmul(
                            O_ps[:, h * 512:(h + 1) * 512], pt_sb,
                            vsrc[:, h * 512:(h + 1) * 512],
                            start=(kt == 0), stop=(kt == L - 1),
                        )
                out_sb = outp.tile([128, D], F32, tag="osb", name=f"ou{i}")
                nc.vector.tensor_scalar_mul(out_sb, O_ps, recip)
                nc.sync.dma_start(out=out_q[i][:, :], in_=out_sb)

            def do_slot(i, ps_pool, s_width, prev):
                L = 2 * (i + 1)
                qt_sb = qt_pool2.tile([128, 8, 128], F32R, tag="qt", name=f"qt{i}")
                nc.gpsimd.dma_start(out=qt_sb, in_=qt_spill[i][:, :, :])
                S_ps = ps_pool.tile([128, s_width], F32, tag="S", name=f"S{i}")
                ngroups = (L * 128 + 511) // 512
                for e in range(8):
                    for kg in range(ngroups):
                        w = min(512, L * 128 - kg * 512)
                        nc.tensor.matmul(
                            S_ps[:, kg * 512: kg * 512 + w],
                            qt_sb[:, e, :],
                            KT[:, e, kg * 512: kg * 512 + w],
                            start=(e == 0), stop=(e == 7),
                        )
                # scores/32 are bounded (|s|/32 <~ 11) -> exp without max-subtraction
                nc.vector.tensor_add(
                    S_ps[:, (L - 2) * 128: L * 128],
                    S_ps[:, (L - 2) * 128: L * 128],
                    mask_sb,
                )
                P_sb = p_hi.tile([128, N], F32R, tag="P", name=f"P{i}")
                stats = sc_pool.tile([128, 4], F32, tag="stats", name=f"st{i}")
                rowsum = stats[:, 2:3]
                nc.scalar.activation(
                    P_sb[:, : L * 128], S_ps[:, : L * 128],
                    mybir.ActivationFunctionType.Exp,
                    bias=0.0, scale=SCALE, accum_out=rowsum,
                )
                recip = stats[:, 3:4]
                nc.vector.reciprocal(recip, rowsum)
                if prev is not None:
                    emit_av(*prev)
                return (i, L, P_sb, recip)

            prev = None
            with tc.tile_pool(name="ps_sA", bufs=2, space="PSUM") as ps_sA:
                for i in range(4):
                    prev = do_slot(i, ps_sA, 1024, prev)
            with tc.tile_pool(name="ps_sB", bufs=1, space="PSUM") as ps_sB:
                for i in range(4, 6):
                    prev = do_slot(i, ps_sB, 2048, prev)
                for j in range(2):
                    vh = vh_pool.tile([128, D], F32R, tag=f"vh{j}", name=f"vh{j}")
                    nc.sync.dma_start(out=vh, in_=v_spill[j][:, :])
                    v_hi.append(vh)
                for i in range(6, N_SLOTS):
                    prev = do_slot(i, ps_sB, 2048, prev)
                emit_av(*prev)

    nc.compile()
    return nc


def _masks():
    q = np.arange(128)[:, None]
    k = np.arange(128)[None, :]
    tril_add = np.where(k <= q, 0.0, NEG).astype(np.float32)
    m0 = np.concatenate([tril_add, np.full((128, 128), NEG, np.float32)], axis=1)
    m1 = np.concatenate([np.zeros((128, 128), np.float32), tril_add], axis=1)
    return m0, m1


def kernel(x, Wq, Wk, Wv):
    global LAST_EXEC_NS
    x = np.ascontiguousarray(np.asarray(x, dtype=np.float32))
    Wq = np.ascontiguousarray(np.asarray(Wq, dtype=np.float32))
    Wk = np.ascontiguousarray(np.asarray(Wk, dtype=np.float32))
    Wv = np.ascontiguousarray(np.asarray(Wv, dtype=np.float32))

    if "nc" not in _NC_CACHE:
        _NC_CACHE["nc"] = _build_nc()
    nc = _NC_CACHE["nc"]

    # host pre-transpose: x[b] (N, D) -> (tile, p=d%128, dchunk, token)
    # element (t, p, c, q) = x[b, t*128+q, c*128+p]
    xt_all = np.ascontiguousarray(
        x.reshape(B, N_KTILES, 128, 8, 128).transpose(0, 1, 4, 3, 2)
    )  # [B, tile, p, c, q]

    # weights host-rearranged to give contiguous per-partition DMA runs
    wq_r = np.ascontiguousarray(Wq.reshape(8, 128, 8, 128).transpose(2, 1, 0, 3))
    wk_r = np.ascontiguousarray(Wk.reshape(8, 128, 8, 128).transpose(2, 1, 0, 3))
    wv_r = np.ascontiguousarray(Wv.reshape(8, 128, 2, 512).transpose(2, 1, 0, 3))

    m0, m1 = _masks()
    in_maps = []
    for c in range(N_CORES):
        b, par = divmod(c, 2)
        in_maps.append({
            "x_t": xt_all[b],
            "x_qt": np.ascontiguousarray(xt_all[b, par::2]),
            "wq": wq_r, "wk": wk_r, "wv": wv_r,
            "mask": m1 if par else m0,
        })

    res = run_bass_kernel_spmd(nc, in_maps, list(range(N_CORES)), trace=TRACE)
    LAST_EXEC_NS = res.exec_time_ns

    out = np.empty((B, N, D), dtype=np.float32)
    for c in range(N_CORES):
        b, par = divmod(c, 2)
        oq = res.results[c]["out_q"]
        for i in range(N_SLOTS):
            g = 2 * i + par
            out[b, g * 128:(g + 1) * 128, :] = oq[i]
    return out



# revision 6
# speedup vs baseline: 1.3876x; 1.3876x over previous
"""Causal attention (B=4, N=2048, D=1024) on 8 Trainium2 NeuronCores.

Sharding: core 2b+r handles batch b, query tiles {r, r+2, ..., r+14}
(128-row tiles, parity-interleaved) — exactly balanced causal split.
K/V projections are deduplicated across the core pair: each core
projects K^T/V only for its own 8 parity-interleaved key tiles, and the
pair exchanges halves with 4 chunked AllGathers (replica groups
[[0,1],[2,3],[4,5],[6,7]]); chunk j delivers global k-tiles 4j..4j+3.

Everything runs in bfloat16 on the PE (fp32 PSUM accumulation), which
enables fast weight loads and halves DMA/SBUF traffic; measured
end-to-end max rel err vs the fp32 reference is ~3e-3 (gate 2e-2).
All operands (x tiles, all weights, K^T, V, Q^T) are SBUF-resident —
no DRAM spills.  Per-core causal masks are passed as input data so the
program is uniform across cores (SPMD).

Schedule: K/V chunks are projected first and fed to the collectives
while the Q projection and early attention hide the exchange latency;
softmax is single-pass (bounded scores) with exp + row-sum fused on the
scalar engine; AV is software-pipelined against the next slot's scores.
"""
import sys

sys.path.insert(0, "/opt/trn_rl_repo")

from contextlib import ExitStack

import numpy as np
import ml_dtypes

import concourse.bass as bass
import concourse.mybir as mybir
import concourse.tile as tile
from concourse import bacc
from concourse.bass_utils import run_bass_kernel_spmd
from concourse.masks import make_identity

B, N, D = 4, 2048, 1024
N_CORES = 8
N_SLOTS = 8          # query tiles per core
N_OWN = 8            # own key tiles per core (pair-deduplicated)
N_KT = 16            # 128-key tiles per batch
NCHUNK = 4           # collective chunks (2 own tiles -> 4 global tiles each)
SCALE = 1.0 / 32.0   # 1/sqrt(D)
NEG = -1.0e9

F32 = mybir.dt.float32
BF16 = mybir.dt.bfloat16
GROUPS = [[0, 1], [2, 3], [4, 5], [6, 7]]

_NC_CACHE = {}
TRACE = False
LAST_EXEC_NS = None


def _build_nc():
    nc = bacc.Bacc(None, target_bir_lowering=False, debug=False)

    # x own tiles, d-major: [own_tile, partition(d%128), dchunk, token]
    x_t = nc.declare_dram_parameter("x_t", [N_OWN, 128, 8, 128], BF16, isOutput=False)
    # weights: wq/wk [echunk, p(d%128), dchunk, ecol]; wv [eh, p, dchunk, ecol]
    wq = nc.declare_dram_parameter("wq", [8, 128, 8, 128], BF16, isOutput=False)
    wk = nc.declare_dram_parameter("wk", [8, 128, 8, 128], BF16, isOutput=False)
    wv = nc.declare_dram_parameter("wv", [2, 128, 8, 512], BF16, isOutput=False)
    mask_in = nc.declare_dram_parameter("mask", [128, 256], F32, isOutput=False)
    out_q = nc.declare_dram_parameter("out_q", [N_SLOTS, 128, D], F32, isOutput=True)

    with tile.TileContext(nc) as tc, ExitStack() as top:
        consts = top.enter_context(tc.tile_pool(name="consts", bufs=1))
        xp = top.enter_context(tc.tile_pool(name="xp", bufs=1))
        wp = top.enter_context(tc.tile_pool(name="wp", bufs=1))
        stg = top.enter_context(tc.tile_pool(name="stg", bufs=1))
        kt_pool = top.enter_context(tc.tile_pool(name="ktp", bufs=1))
        v_pool = top.enter_context(tc.tile_pool(name="vp", bufs=1))
        qt_pool = top.enter_context(tc.tile_pool(name="qtp", bufs=1))
        dram = top.enter_context(tc.tile_pool(name="dram", bufs=1, space="DRAM"))

        ident_f = consts.tile([128, 128], F32)
        make_identity(nc, ident_f)
        ident = consts.tile([128, 128], BF16)
        nc.vector.tensor_copy(ident, ident_f)
        mask_sb = consts.tile([128, 256], F32)
        nc.sync.dma_start(out=mask_sb, in_=mask_in[:, :])

        # tiny dummy collective to absorb first-CC setup latency
        dummy_in = dram.tile([128, 8], BF16, name="cc_dummy_in")
        dummy_out = dram.tile([2, 128, 8], BF16, name="cc_dummy_out")
        zsb = consts.tile([128, 8], BF16)
        nc.gpsimd.memset(zsb, 0.0)
        nc.gpsimd.dma_start(out=dummy_in[:], in_=zsb)
        nc.gpsimd.collective_compute(
            "AllGather", mybir.AluOpType.bypass, replica_groups=GROUPS,
            ins=[dummy_in.opt()], outs=[dummy_out.opt()],
        )

        X = xp.tile([128, N_OWN, 8, 128], BF16)          # [p, own_tile, c, q]
        for i in range(N_OWN):
            nc.sync.dma_start(out=X[:, i, :, :], in_=x_t[i][:, :, :])

        wv_sb = wp.tile([128, 2, 8, 512], BF16)
        for eh in range(2):
            for h2 in range(2):
                nc.scalar.dma_start(
                    out=wv_sb[:, eh, h2 * 4:(h2 + 1) * 4, :],
                    in_=wv[eh][:, h2 * 4:(h2 + 1) * 4, :],
                )
        wk_sb = wp.tile([128, 8, 8, 128], BF16)
        wq_sb = wp.tile([128, 8, 8, 128], BF16)
        for e in range(8):
            nc.scalar.dma_start(out=wk_sb[:, e, :, :], in_=wk[e][:, :, :])
        for e in range(8):
            nc.scalar.dma_start(out=wq_sb[:, e, :, :], in_=wq[e][:, :, :])

        # staging for own-half projections (own-local layout, rank-uniform)
        KTstg = stg.tile([128, N_OWN, 8, 128], BF16)     # [p(e%128), i, echunk, key]
        Vstg = stg.tile([128, N_OWN, D], BF16)           # [p(k%128), i, e]
        KT = kt_pool.tile([128, 8, N], BF16)             # [p(e%128), echunk, global key]
        V = v_pool.tile([128, N_KT, D], BF16)            # [p(k%128), global tile, e]
        QT = qt_pool.tile([128, 8, N_SLOTS, 128], BF16)  # [p(e%128), echunk, slot, q]

        bin_t = [dram.tile([128, 4096], BF16, name=f"ccin{j}") for j in range(NCHUNK)]
        bout_t = [dram.tile([2, 128, 4096], BF16, name=f"ccout{j}") for j in range(NCHUNK)]

        with ExitStack() as ph1:
            ps_mm = ph1.enter_context(tc.tile_pool(name="ps_mm", bufs=8, space="PSUM"))

            def v_tile(i):
                for eh in range(2):
                    vps = ps_mm.tile([128, 512], F32, tag="mm", name=f"v{i}_{eh}")
                    for c in range(8):
                        nc.tensor.matmul(
                            vps, X[:, i, c, :], wv_sb[:, eh, c, :],
                            start=(c == 0), stop=(c == 7),
                        )
                    nc.vector.tensor_copy(Vstg[:, i, eh * 512:(eh + 1) * 512], vps)

            def k_group(kg):
                for e in range(8):
                    kps = ps_mm.tile([128, 512], F32, tag="mm", name=f"k{kg}_{e}")
                    for c in range(8):
                        nc.tensor.matmul(
                            kps, wk_sb[:, e, c, :], X[:, kg * 4:(kg + 1) * 4, c, :],
                            start=(c == 0), stop=(c == 7),
                        )
                    nc.vector.tensor_copy(
                        KTstg[:, kg * 4:(kg + 1) * 4, e, :],
                        kps.rearrange("p (i q) -> p i q", i=4),
                    )

            def send_chunk(j):
                nc.scalar.dma_start(
                    out=bin_t[j][:, 0:2048],
                    in_=KTstg[:, 2 * j:2 * j + 2, :, :].rearrange("p i e q -> p (i e q)"),
                )
                nc.scalar.dma_start(
                    out=bin_t[j][:, 2048:4096],
                    in_=Vstg[:, 2 * j:2 * j + 2, :].rearrange("p i e -> p (i e)"),
                )
                nc.gpsimd.collective_compute(
                    "AllGather", mybir.AluOpType.bypass, replica_groups=GROUPS,
                    ins=[bin_t[j].opt()], outs=[bout_t[j].opt()],
                )

            def recv_chunk(j):
                for rk in range(2):
                    for i2 in range(2):
                        g = 4 * j + 2 * i2 + rk
                        nc.sync.dma_start(
                            out=KT[:, :, g * 128:(g + 1) * 128],
                            in_=bout_t[j][rk][:, i2 * 1024:(i2 + 1) * 1024]
                            .rearrange("p (e q) -> p e q", e=8),
                        )
                        nc.sync.dma_start(
                            out=V[:, g, :],
                            in_=bout_t[j][rk][:, 2048 + i2 * 1024:2048 + (i2 + 1) * 1024],
                        )

            k_group(0)
            v_tile(0); v_tile(1)
            send_chunk(0)
            v_tile(2); v_tile(3)
            send_chunk(1)
            k_group(1)
            v_tile(4); v_tile(5)
            send_chunk(2)
            v_tile(6); v_tile(7)
            send_chunk(3)

            def q_group(qg):
                for e in range(8):
                    qps = ps_mm.tile([128, 512], F32, tag="mm", name=f"q{qg}_{e}")
                    for c in range(8):
                        nc.tensor.matmul(
                            qps, wq_sb[:, e, c, :], X[:, qg * 4:(qg + 1) * 4, c, :],
                            start=(c == 0), stop=(c == 7),
                        )
                    nc.vector.tensor_copy(
                        QT[:, e, qg * 4:(qg + 1) * 4, :],
                        qps.rearrange("p (s q) -> p s q", s=4),
                    )

            q_group(0)
            q_group(1)
            for j in range(NCHUNK):
                recv_chunk(j)

        # ---- attention slots 0-7, software-pipelined AV ----
        with ExitStack() as ph3:
            ps_tr = ph3.enter_context(tc.tile_pool(name="ps_tr", bufs=2, space="PSUM"))
            ps_o = ph3.enter_context(tc.tile_pool(name="ps_o", bufs=1, space="PSUM"))
            p_hi = ph3.enter_context(tc.tile_pool(name="phi", bufs=2))
            pt_pool = ph3.enter_context(tc.tile_pool(name="ptp", bufs=2))
            sc_pool = ph3.enter_context(tc.tile_pool(name="scp", bufs=2))
            outp = ph3.enter_context(tc.tile_pool(name="outp", bufs=2))

            def emit_av(i, L, P_sb, recip):
                O_ps = ps_o.tile([128, D], F32, tag="O", name=f"O{i}")
                for kt in range(L):
                    ptps = ps_tr.tile([128, 128], BF16, tag="tr", name=f"tp{i}_{kt}")
                    nc.tensor.transpose(ptps, P_sb[:, kt * 128:(kt + 1) * 128], ident)
                    pt_sb = pt_pool.tile([128, 128], BF16, tag="pts", name=f"pt{i}_{kt}")
                    nc.vector.tensor_copy(pt_sb, ptps)
                    for h in range(2):
                        nc.tensor.matmul(
                            O_ps[:, h * 512:(h + 1) * 512], pt_sb,
                            V[:, kt, h * 512:(h + 1) * 512],
                            start=(kt == 0), stop=(kt == L - 1),
                        )
                out_sb = outp.tile([128, D], F32, tag="osb", name=f"ou{i}")
                nc.vector.tensor_scalar_mul(out_sb, O_ps, recip)
                nc.scalar.dma_start(out=out_q[i][:, :], in_=out_sb)

            def do_slot(i, ps_pool, s_width, prev):
                L = 2 * (i + 1)
                S_ps = ps_pool.tile([128, s_width], F32, tag="S", name=f"S{i}")
                ngroups = (L * 128 + 511) // 512
                for e in range(8):
                    for kg in range(ngroups):
                        w = min(512, L * 128 - kg * 512)
                        nc.tensor.matmul(
                            S_ps[:, kg * 512: kg * 512 + w],
                            QT[:, e, i, :],
                            KT[:, e, kg * 512: kg * 512 + w],
                            start=(e == 0), stop=(e == 7),
                        )
                # scores/32 are bounded (|s|/32 <~ 11) -> exp without max-subtraction
                nc.vector.tensor_add(
                    S_ps[:, (L - 2) * 128: L * 128],
                    S_ps[:, (L - 2) * 128: L * 128],
                    mask_sb,
                )
                P_sb = p_hi.tile([128, N], BF16, tag="P", name=f"P{i}")
                stats = sc_pool.tile([128, 4], F32, tag="stats", name=f"st{i}")
                rowsum = stats[:, 2:3]
                nc.scalar.activation(
                    P_sb[:, : L * 128], S_ps[:, : L * 128],
                    mybir.ActivationFunctionType.Exp,
                    bias=0.0, scale=SCALE, accum_out=rowsum,
                )
                recip = stats[:, 3:4]
                nc.vector.reciprocal(recip, rowsum)
                if prev is not None:
                    emit_av(*prev)
                return (i, L, P_sb, recip)

            prev = None
            with tc.tile_pool(name="ps_sA", bufs=2, space="PSUM") as ps_sA:
                for i in range(4):
                    prev = do_slot(i, ps_sA, 1024, prev)
            with tc.tile_pool(name="ps_sB", bufs=1, space="PSUM") as ps_sB:
                for i in range(4, N_SLOTS):
                    prev = do_slot(i, ps_sB, 2048, prev)
                emit_av(*prev)

    nc.compile()
    return nc


def _masks():
    q = np.arange(128)[:, None]
    k = np.arange(128)[None, :]
    tril_add = np.where(k <= q, 0.0, NEG).astype(np.float32)
    m0 = np.concatenate([tril_add, np.full((128, 128), NEG, np.float32)], axis=1)
    m1 = np.concatenate([np.zeros((128, 128), np.float32), tril_add], axis=1)
    return m0, m1


def kernel(x, Wq, Wk, Wv):
    global LAST_EXEC_NS
    x = np.ascontiguousarray(np.asarray(x, dtype=np.float32))
    Wq = np.ascontiguousarray(np.asarray(Wq, dtype=np.float32))
    Wk = np.ascontiguousarray(np.asarray(Wk, dtype=np.float32))
    Wv = np.ascontiguousarray(np.asarray(Wv, dtype=np.float32))

    if "nc" not in _NC_CACHE:
        _NC_CACHE["nc"] = _build_nc()
    nc = _NC_CACHE["nc"]

    bf = ml_dtypes.bfloat16
    # host pre-transpose: x[b] (N, D) -> (tile, p=d%128, dchunk, token)
    xt_all = np.ascontiguousarray(
        x.reshape(B, N_KT, 128, 8, 128).transpose(0, 1, 4, 3, 2).astype(bf)
    )  # [B, tile, p, c, q]

    wq_r = np.ascontiguousarray(Wq.reshape(8, 128, 8, 128).transpose(2, 1, 0, 3).astype(bf))
    wk_r = np.ascontiguousarray(Wk.reshape(8, 128, 8, 128).transpose(2, 1, 0, 3).astype(bf))
    wv_r = np.ascontiguousarray(Wv.reshape(8, 128, 2, 512).transpose(2, 1, 0, 3).astype(bf))

    m0, m1 = _masks()
    in_maps = []
    for c in range(N_CORES):
        b, par = divmod(c, 2)
        in_maps.append({
            "x_t": np.ascontiguousarray(xt_all[b, par::2]),
            "wq": wq_r, "wk": wk_r, "wv": wv_r,
            "mask": m1 if par else m0,
        })

    res = run_bass_kernel_spmd(nc, in_maps, list(range(N_CORES)), trace=TRACE)
    LAST_EXEC_NS = res.exec_time_ns

    out = np.empty((B, N, D), dtype=np.float32)
    for c in range(N_CORES):
        b, par = divmod(c, 2)
        oq = res.results[c]["out_q"]
        for i in range(N_SLOTS):
            g = 2 * i + par
            out[b, g * 128:(g + 1) * 128, :] = oq[i]
    return out
